# revision 33
# baseline (speedup 1.0000x reference)
"""Trainium2 Bass kernel for CachedHeavyRecentAttentionMasker.

Problem shape: scores (1, 32, 512, 4096) f32, group_size=4.
k = 819 (top-k per row), threshold = 1638 (union-size stop test).

Algorithm notes
---------------
The reference module (torch original) walks query rows back-to-front,
unioning each row's top-k key mask per head, and stops as soon as every
head's running union reaches `threshold`. For randn scores the union of n
rows has expected size Lk*(1-0.8^n), so the stop fires at n ~ 3; only a
handful of trailing rows are ever needed. We exploit exactly that laziness:
the device computes exact per-row top-k membership masks for the trailing
CHUNK=32 query rows of every head (sharded 4 heads/core across 8 cores),
and the host performs the tiny union/threshold bookkeeping on those masks.
If 32 trailing rows ever turn out to be insufficient (statistically never
for this regime), we fall back to an exact full numpy implementation.

Device kernel (per core, [128 rows x 4096] f32, one row per partition):
bisection on the top-k threshold value. Each probe is a single fused DVE
tensor_scalar (is_ge against a per-partition scalar, with accumulated
free-dim reduce -> per-row count), followed by two [128,1] update ops.
12 probes narrow the bracket (0.725, 0.975) - a >6-sigma cover of the
randn k-th order statistic - to ~6e-5, below the typical gap between the
k-th and (k+1)-th order statistic (~9e-4). The final full-row pass writes
the mask {x >= t}; the host accepts a row iff its popcount == k, which
guarantees the mask is exactly the stable top-k set, and recomputes the
few straddled rows (~3%) exactly in numpy.
"""

import sys

import numpy as np

_TRN_REPO = "/opt/trn_rl_repo"
if _TRN_REPO not in sys.path:
    sys.path.insert(0, _TRN_REPO)

# Problem constants (hardcoded per harness contract).
B, H, LQ, LK = 1, 32, 512, 4096
N_CORES = 8
HEADS_PER_CORE = H // N_CORES  # 4
CHUNK = 32  # trailing query rows per head handled by the device
ROWS_PER_CORE = HEADS_PER_CORE * CHUNK  # 128
HEAVY_BUDGET_RATIO = 0.2
REBUILD_CAP_RATIO = 0.75

N_PROBES = 12
# Probe columns counted by the scalar engine (0 = DVE only). The split
# models ~10% faster on the engine cost model, but costs 2 extra
# instructions + 2 cross-engine syncs per probe; single-engine is faster
# and far more predictable when per-instruction overhead is nontrivial.
F_ACT = 0
# Initial bisection bracket (center, half-width). The k-th order statistic
# of a randn row sits at the 0.8 quantile, 0.8416 +- ~0.022; (0.725, 0.975)
# covers it beyond 6 sigma. Rows that escape the bracket (adversarial
# inputs) simply fail the count==k check and take the exact host path.
BRACKET_MID = 0.85
BRACKET = 0.125

_NC = None
LAST_RESULTS = None


def _build_nc(n_probes=N_PROBES, f_act=F_ACT):
    import concourse.bacc as bacc
    import concourse.mybir as mybir
    import concourse.tile as tile

    op = mybir.AluOpType
    f32 = mybir.dt.float32

    k = max(1, min(int(HEAVY_BUDGET_RATIO * LK), LK))
    f_dve = LK - f_act  # DVE share of each probe row; ACT counts the rest

    nc = bacc.Bacc()
    x_d = nc.dram_tensor("rows", [ROWS_PER_CORE, LK], f32, kind="ExternalInput")
    # Top-k membership mask (0/1) per row. The host validates each row by
    # popcount == k. bf16: an 8-bit output dtype would drop the final DVE
    # pass to 1x mode.
    mask_d = nc.dram_tensor(
        "mask", [ROWS_PER_CORE, LK], mybir.dt.bfloat16, kind="ExternalOutput"
    )

    with tile.TileContext(nc) as tc:
        with (
            tc.tile_pool(name="data", bufs=1) as data_pool,
            tc.tile_pool(name="small", bufs=1) as small,
        ):
            x = data_pool.tile([ROWS_PER_CORE, LK], f32)
            # one 128-partition 2MiB transfer: InstDMACopy fans out across
            # all 16 SDMA engines; splitting would serialize on the same ring
            nc.sync.dma_start(x[:], x_d[:, :])

            scr = data_pool.tile([ROWS_PER_CORE, LK], f32)
            mask = data_pool.tile([ROWS_PER_CORE, LK], mybir.dt.bfloat16)
            mid = small.tile([ROWS_PER_CORE, 1], f32)
            cnt = small.tile([ROWS_PER_CORE, 1], f32)
            d = small.tile([ROWS_PER_CORE, 1], f32)
            if f_act:
                scr_act = data_pool.tile([ROWS_PER_CORE, f_act], mybir.dt.bfloat16)
                s2 = small.tile([ROWS_PER_CORE, 1], f32)

            # mid = BRACKET_MID, expressed as 0*x + c so the input-DMA wait
            # attaches HERE: the TensorScalarPtr ISA struct has a single
            # sync-wait slot, so the first probe must depend on the DMA only
            # transitively through the DVE sem chain.
            nc.vector.tensor_scalar(
                out=mid[:],
                in0=x[:, 0:1],
                scalar1=0.0,
                scalar2=BRACKET_MID,
                op0=op.mult,
                op1=op.add,
            )
            if f_act:
                # ACT warmup touching x: absorbs the input-DMA wait so the
                # first real ACT probe needs only its DVE-sem wait.
                nc.scalar.copy(scr_act[:, 0:1], x[:, 0:1])

            afn = mybir.ActivationFunctionType
            c = BRACKET
            for i in range(n_probes):
                last = i == n_probes - 1
                # DVE share: cnt[r] = #{ x[r, :f_dve] >= mid[r] }
                nc.vector.tensor_scalar(
                    out=scr[:, 0:f_dve],
                    in0=x[:, 0:f_dve],
                    scalar1=mid[:],
                    scalar2=None,
                    op0=op.is_ge,
                    op1=op.add,
                    accum_out=cnt[:],
                )
                if f_act:
                    # ACT share: s2[r] = sum sign(mid[r] - x[r, f_dve:])
                    #          = (#lt - #gt) over the ACT columns
                    nc.scalar.activation(
                        out=scr_act[:],
                        in_=x[:, f_dve:LK],
                        func=afn.Sign,
                        bias=mid[:],
                        scale=-1.0,
                        accum_out=s2[:],
                    )
                c *= 0.5
                if f_act:
                    # total count >= k  <=>  2*cnt - s2 >= 2k - f_act
                    # (exact when no probe value collides with a data value;
                    # a collision only perturbs the search path and is caught
                    # by the final count==k check)
                    nc.vector.tensor_scalar(
                        out=d[:],
                        in0=cnt[:],
                        scalar1=2.0,
                        op0=op.mult,
                        scalar2=s2[:],
                        op1=op.subtract,
                    )
                    nc.vector.tensor_scalar(
                        out=d[:],
                        in0=d[:],
                        scalar1=float(2 * k - f_act) - 0.5,
                        op0=op.is_ge,
                        scalar2=2.0 * c,
                        op1=op.mult,
                    )
                else:
                    # d = (cnt >= k-0.5) * 2c ; mid = (mid - c) + d => mid +- c
                    nc.vector.tensor_scalar(
                        out=d[:],
                        in0=cnt[:],
                        scalar1=float(k) - 0.5,
                        scalar2=2.0 * c,
                        op0=op.is_ge,
                        op1=op.mult,
                    )
                # mid = (mid - c) + d => mid +- c. On the last probe fold in
                # the extra -c so mid lands directly on the final bracket's
                # lower edge t (count(x >= t) >= k).
                nc.vector.tensor_scalar(
                    out=mid[:],
                    in0=mid[:],
                    scalar1=-2.0 * c if last else -c,
                    scalar2=d[:],
                    op0=op.add,
                    op1=op.add,
                )
            nc.vector.tensor_scalar(
                out=mask[:],
                in0=x[:],
                scalar1=mid[:],
                scalar2=None,
                op0=op.is_ge,
                op1=op.add,
                accum_out=cnt[:],
            )
            nc.sync.dma_start(mask_d[:, :], mask[:])
    nc.compile()
    return nc


def _get_nc():
    global _NC
    if _NC is None:
        _NC = _build_nc()
    return _NC


def _topk_mask_row_exact(row, k):
    """Exact stable top-k membership mask (jax.lax.top_k tie semantics:
    equal values resolved by lowest index)."""
    lk = row.shape[0]
    t = np.partition(row, lk - k)[lk - k]  # k-th largest value
    mask = row > t
    m = k - int(mask.sum())
    if m > 0:
        eq_idx = np.nonzero(row == t)[0][:m]
        mask[eq_idx] = True
    return mask


def _host_full_reference(scores, group_size):
    """Exact numpy port of the reference for the (statistically impossible)
    case where 32 trailing rows don't settle every head, or for non-standard
    shapes."""
    b, h, lq, lk = scores.shape
    k = max(1, min(int(HEAVY_BUDGET_RATIO * lk), lk))
    threshold = max(1, min(2 * k, int(REBUILD_CAP_RATIO * lk)))

    masks = np.zeros((b, h, lq, lk), dtype=bool)
    for bi in range(b):
        for hi in range(h):
            for qi in range(lq):
                masks[bi, hi, qi] = _topk_mask_row_exact(scores[bi, hi, qi], k)

    flipped = masks[:, :, ::-1, :]
    cum = np.logical_or.accumulate(flipped, axis=2)
    sizes = cum.sum(axis=-1)
    reached = sizes >= threshold
    chosen = reached.any(axis=-1)
    n_sel = np.where(chosen, np.argmax(reached, axis=-1) + 1, 1)
    n_stop = n_sel.max() if chosen.all() else lq
    final_union = cum[:, :, n_stop - 1, :]
    return _assemble_output(scores, final_union, group_size)


def _assemble_output(scores, final_union, group_size):
    b, h, lq, lk = scores.shape
    final_last = final_union
    if group_size > 1:
        g = h // group_size
        gu = final_last.reshape(b, g, group_size, lk).any(axis=2)
        final_last = np.broadcast_to(
            gu[:, :, None, :], (b, g, group_size, lk)
        ).reshape(b, h, lk)
    min_value = np.finfo(np.float32).min
    last_row = np.where(final_last, np.float32(0.0), min_value).astype(np.float32)
    sparse = np.zeros((b, h, lq, lk), dtype=np.float32)
    sparse[:, :, -1, :] = last_row
    density = np.float32(final_last.astype(np.float32).mean())
    return sparse, density


def kernel(**inputs):
    global LAST_RESULTS
    scores = np.ascontiguousarray(
        np.asarray(inputs["scores_plus_mask_4d"], dtype=np.float32)
    )
    group_size = int(np.asarray(inputs["group_size"]))

    b, h, lq, lk = scores.shape
    if (b, h, lq, lk) != (B, H, LQ, LK):
        return _host_full_reference(scores, group_size)

    k = max(1, min(int(HEAVY_BUDGET_RATIO * lk), lk))
    threshold = max(1, min(2 * k, int(REBUILD_CAP_RATIO * lk)))

    from concourse.bass_utils import run_bass_kernel_spmd

    nc = _get_nc()
    # Core c handles heads [4c, 4c+4), trailing CHUNK query rows, natural
    # (ascending q) order: partition p = head_local*CHUNK + (q - (LQ-CHUNK)).
    tail = scores[0, :, LQ - CHUNK :, :]  # (H, CHUNK, LK)
    in_maps = [
        {
            "rows": np.ascontiguousarray(
                tail[c * HEADS_PER_CORE : (c + 1) * HEADS_PER_CORE].reshape(
                    ROWS_PER_CORE, LK
                )
            )
        }
        for c in range(N_CORES)
    ]
    res = run_bass_kernel_spmd(nc, in_maps, core_ids=list(range(N_CORES)))
    LAST_RESULTS = res

    # (H, CHUNK, LK) bf16 0/1, ascending q within each head
    masks = (
        np.concatenate(
            [
                res.results[c]["mask"].reshape(HEADS_PER_CORE, CHUNK, LK)
                for c in range(N_CORES)
            ],
            axis=0,
        )
        != 0
    )
    # A row is device-exact iff its mask has exactly k members: the mask is
    # {x >= t}, so popcount == k <=> it is exactly the stable top-k set.
    ok = masks.sum(axis=-1) == k

    # Host fixup for rows whose final bracket still straddles another data
    # value (count != k). Vectorized exact top-k over just those rows.
    bad = np.argwhere(~ok)
    if len(bad):
        rows = np.stack(
            [scores[0, hi, LQ - CHUNK + ji] for hi, ji in bad]
        )  # (nbad, LK)
        t = np.partition(rows, LK - k, axis=1)[:, LK - k]  # k-th largest
        gt = rows > t[:, None]
        need = k - gt.sum(axis=1)
        eq_rank = np.cumsum(rows == t[:, None], axis=1)
        fixed = gt | ((rows == t[:, None]) & (eq_rank <= need[:, None]))
        for row_idx, (hi, ji) in enumerate(bad):
            masks[hi, ji] = fixed[row_idx]

    # Union scan back-to-front over the trailing CHUNK rows.
    flipped = masks[:, ::-1, :]  # n-1 index == union depth n
    cum = np.logical_or.accumulate(flipped, axis=1)
    sizes = cum.sum(axis=-1)  # (H, CHUNK)
    reached = sizes >= threshold
    per_head_any = reached.any(axis=-1)
    if not per_head_any.all():
        return _host_full_reference(scores, group_size)

    n_sel = np.argmax(reached, axis=-1) + 1  # first n with |union| >= threshold
    n_stop = int(n_sel.max())
    final_union = cum[np.arange(H), n_stop - 1][None]  # (1, H, LK)
    return _assemble_output(scores, final_union, group_size)
